# revision 4
# baseline (speedup 1.0000x reference)
"""Trainium2 Bass kernel for nn_DenseGraphConvEdgeToEdge (B=4, N=256, C=O=128).

out[b,i,j,:] = E[b,i,j]@W0 + E[b,j,i]@W1 + R[b,i]@W2 + Cm[b,j]@W3
             + R[b,j]@W4 + Cm[b,i]@W5 + sa[b]@W6 + bias
where R = E.sum(axis=2) (row sums), Cm = E.sum(axis=1) (col sums),
sa = E.sum(axis=(1,2)).

Sharding: 8 cores = 4 batches x 2 halves. Core (b, h) owns output quadrants
qA=(0,h), qB=(1,1-h) (quadrant (p,q) = rows p*128:(p+1)*128 x cols
q*128:(q+1)*128) and loads exactly the matching E quadrants, pre-transposed
on the host to [c, i*128+j] layout. The E->E transpose term for quadrant
(p,q) reads quadrant (q,p) via a strided in-SBUF access pattern: for h=0
(diagonal quads) that's the same tile, for h=1 the sibling tile. The program
is SPMD-uniform: both candidate transpose reads are always issued, with
host-provided weights W1 or 0 selecting the applicable one.

Marginals: each core computes per-quadrant partial row/col sums, exchanges
them with its batch-sibling via a pairwise AllGather (the only collective),
then forms the broadcast terms G (per output column) and P (per output row)
via matmuls against host-built selector weight tables. G (+ sa@W6 + bias) is
folded into the PSUM accumulation with an [I|I] identity matmul; P is added
during the PSUM->SBUF drain (per-partition scalar add).

Matmuls run as float32r (TF32-like, 1 cycle/row at N>=256 vs 4 for fp32);
end-to-end rel err ~1.5e-4.
"""
import numpy as np

import concourse.mybir as mybir
import concourse.tile as tile
from concourse import bacc
from concourse.bass_utils import run_bass_kernel_spmd

F32 = mybir.dt.float32
F32R = mybir.dt.float32r
F16 = mybir.dt.float16
ADD = mybir.AluOpType.add
MM_DT = F16          # dtype for the big E-term matmuls (F16 or F32R)
E_NP = np.float16 if MM_DT == F16 else np.float32

B, N, C, O = 4, 256, 128, 128
Q = 128          # quadrant side
QF = Q * Q       # quadrant flat free size
N_CORES = 8

_NC_CACHE = {}


def _sel(w, cond):
    return w if cond else np.zeros_like(w)


def build(use_collective=True):
    nc = bacc.Bacc(trn_type="TRN2")

    # per-core inputs
    eqA = nc.dram_tensor("eqA", [C, QF], MM_DT, kind="ExternalInput")
    eqB = nc.dram_tensor("eqB", [C, QF], MM_DT, kind="ExternalInput")
    wsb_d = nc.dram_tensor("wsb", [C, 7 * O], F32, kind="ExternalInput")
    w0_d = nc.dram_tensor("w0m", [C, O], MM_DT, kind="ExternalInput")
    i2_d = nc.dram_tensor("i2", [C, 4 * Q], MM_DT, kind="ExternalInput")
    w1a_d = nc.dram_tensor("w1a", [C, O], MM_DT, kind="ExternalInput")
    w1b_d = nc.dram_tensor("w1b", [C, O], MM_DT, kind="ExternalInput")
    gselA_d = nc.dram_tensor("gselA", [C, 8 * O], F32, kind="ExternalInput")
    gselB_d = nc.dram_tensor("gselB", [C, 8 * O], F32, kind="ExternalInput")
    pselA_d = nc.dram_tensor("pselA", [C, 8 * O], F32, kind="ExternalInput")
    pselB_d = nc.dram_tensor("pselB", [C, 8 * O], F32, kind="ExternalInput")
    biasr_d = nc.dram_tensor("biasr", [1, O], F32, kind="ExternalInput")
    ones_d = nc.dram_tensor("ones", [1, Q], F32, kind="ExternalInput")
    outA = nc.dram_tensor("outA", [O, QF], F32, kind="ExternalOutput")
    outB = nc.dram_tensor("outB", [O, QF], F32, kind="ExternalOutput")

    with tile.TileContext(nc) as tc:
        with (
            tc.tile_pool(name="pool", bufs=1) as pool,
            tc.tile_pool(name="stpool", bufs=3) as stpool,
            tc.tile_pool(name="ppmain", bufs=5, space="PSUM") as ppmain,
            tc.tile_pool(name="ppaux", bufs=3, space="PSUM") as ppaux,
            tc.tile_pool(name="dram", bufs=1, space="DRAM") as dram,
        ):
            # ---- small constant loads ----
            wsb = pool.tile([C, 7 * O], F32R, tag="wsb")
            nc.sync.dma_start(wsb[:], wsb_d[:].bitcast(F32R))
            i2t = pool.tile([C, 4 * Q], MM_DT, tag="i2t")
            nc.sync.dma_start(i2t[:], i2_d[:] if MM_DT == F16 else i2_d[:].bitcast(F32R))
            w0m = pool.tile([C, O], MM_DT, tag="w0m")
            nc.sync.dma_start(w0m[:], w0_d[:] if MM_DT == F16 else w0_d[:].bitcast(F32R))
            w1a = pool.tile([C, O], MM_DT, tag="w1a")
            nc.sync.dma_start(w1a[:], w1a_d[:] if MM_DT == F16 else w1a_d[:].bitcast(F32R))
            w1b = pool.tile([C, O], MM_DT, tag="w1b")
            nc.sync.dma_start(w1b[:], w1b_d[:] if MM_DT == F16 else w1b_d[:].bitcast(F32R))
            gselA = pool.tile([C, 8 * O], F32R, tag="gselA")
            nc.sync.dma_start(gselA[:], gselA_d[:].bitcast(F32R))
            gselB = pool.tile([C, 8 * O], F32R, tag="gselB")
            nc.sync.dma_start(gselB[:], gselB_d[:].bitcast(F32R))
            pselA = pool.tile([C, 8 * O], F32R, tag="pselA")
            nc.sync.dma_start(pselA[:], pselA_d[:].bitcast(F32R))
            pselB = pool.tile([C, 8 * O], F32R, tag="pselB")
            nc.sync.dma_start(pselB[:], pselB_d[:].bitcast(F32R))
            biasr = pool.tile([1, O], F32, tag="biasr")
            nc.sync.dma_start(biasr[:], biasr_d[:])
            onesr = pool.tile([1, Q], F32R, tag="onesr")
            nc.sync.dma_start(onesr[:], ones_d[:].bitcast(F32R))

            # ---- resident E quadrants, chunked loads ----
            rtA = pool.tile([C, QF], MM_DT, tag="rtA")
            rtB = pool.tile([C, QF], MM_DT, tag="rtB")
            NCHUNK = 8
            CH = QF // NCHUNK  # 2048
            for rt, src in ((rtA, eqA), (rtB, eqB)):
                for k in range(NCHUNK):
                    sl = slice(k * CH, (k + 1) * CH)
                    nc.sync.dma_start(rt[:, sl], src[:, sl] if MM_DT == F16 else src[:, sl].bitcast(F32R))

            # ---- per-quadrant marginal partials ----
            # own_pack = [prA | pcmA | prB | pcmB], each [c, 128]
            own_pack = pool.tile([C, 512], F32, tag="own_pack")
            ident = i2t[:, 0:Q]
            for qi, rt in enumerate((rtA, rtB)):
                # col sums via identity-matmul accumulation over i-pairs
                ps_cm = ppaux.tile([C, 2 * Q], F32, tag="cm", name=f"pscm{qi}")
                for t in range(Q // 2):
                    nc.tensor.matmul(ps_cm[:], ident, rt[:, t * 256:(t + 1) * 256],
                                     start=(t == 0), stop=(t == Q // 2 - 1))
                tmp = pool.tile([C, Q], F32, tag="cmtmp", name=f"cmtmp{qi}")
                nc.vector.tensor_copy(tmp[:], ps_cm[:, 0:Q])
                nc.vector.tensor_tensor(own_pack[:, (2 * qi + 1) * Q:(2 * qi + 2) * Q],
                                        tmp[:], ps_cm[:, Q:2 * Q], op=ADD)
                # row sums via DVE reduce, chunked 16 rows at a time
                rt3 = (rt[:] if MM_DT == F16 else rt[:].bitcast(F32)).rearrange("c (i j) -> c i j", i=Q)
                for k in range(NCHUNK):
                    nc.vector.tensor_reduce(
                        own_pack[:, 2 * qi * Q + k * 16:2 * qi * Q + (k + 1) * 16],
                        rt3[:, k * 16:(k + 1) * 16, :],
                        axis=mybir.AxisListType.X, op=ADD)

            # ---- exchange partials with batch sibling ----
            cc_in = dram.tile([C, 512], F32, tag="cc_in")
            cc_out = dram.tile([2 * C, 512], F32, tag="cc_out")
            nc.gpsimd.dma_start(cc_in[:], own_pack[:])
            if use_collective:
                nc.gpsimd.collective_compute(
                    "AllGather", mybir.AluOpType.bypass,
                    replica_groups=[[0, 1], [2, 3], [4, 5], [6, 7]],
                    ins=[cc_in[:].opt()], outs=[cc_out[:].opt()])
            else:
                nc.gpsimd.dma_start(cc_out[0:C, :], cc_in[:])
                nc.gpsimd.dma_start(cc_out[C:2 * C, :], cc_in[:])
            pk0 = pool.tile([C, 512], F32R, tag="pk0")
            nc.sync.dma_start(pk0[:], cc_out[0:C, :].bitcast(F32R))
            pk1 = pool.tile([C, 512], F32R, tag="pk1")
            nc.sync.dma_start(pk1[:], cc_out[C:2 * C, :].bitcast(F32R))

            # ---- sa (sum over everything); pack totals = 2*sa ----
            sa0 = pool.tile([C, 1], F32, tag="sa0")
            nc.vector.tensor_reduce(sa0[:], pk0[:].bitcast(F32),
                                    axis=mybir.AxisListType.X, op=ADD)
            sa1 = pool.tile([C, 1], F32, tag="sa1")
            nc.vector.tensor_reduce(sa1[:], pk1[:].bitcast(F32),
                                    axis=mybir.AxisListType.X, op=ADD)
            sa2 = pool.tile([C, 1], F32, tag="sa2")
            nc.vector.tensor_tensor(sa2[:], sa0[:], sa1[:], op=ADD)
            saT = pool.tile([C, 1], F32R, tag="saT")
            nc.vector.tensor_copy(saT[:], sa2[:].bitcast(F32R))

            # ---- s = (2*sa) @ (W6/2) + bias, as a [1, O] row ----
            ps_s = ppaux.tile([1, O], F32, tag="cm", name="ps_s")
            nc.tensor.matmul(ps_s[:], saT[:], wsb[:, 6 * O:7 * O],
                             start=True, stop=True)
            sbrow = pool.tile([1, O], F32, tag="sbrow")
            nc.vector.tensor_tensor(sbrow[:], biasr[:], ps_s[:], op=ADD)
            sbrow_r = pool.tile([1, O], F32R, tag="sbrow_r")
            nc.vector.tensor_copy(sbrow_r[:], sbrow[:].bitcast(F32R))

            # ---- G tiles (per-output-column broadcast, [j, o]) ----
            srcs = [pk0[:, k * Q:(k + 1) * Q] for k in range(4)] + \
                   [pk1[:, k * Q:(k + 1) * Q] for k in range(4)]
            g_sb = []
            for name, gsel in (("ga", gselA), ("gb", gselB)):
                ps_g = ppaux.tile([Q, O], F32, tag="cm", name=f"psg_{name}")
                for k, s in enumerate(srcs):
                    nc.tensor.matmul(ps_g[:], s, gsel[:, k * O:(k + 1) * O],
                                     start=(k == 0), stop=False)
                nc.tensor.matmul(ps_g[:], onesr[:], sbrow_r[:],
                                 start=False, stop=True)
                gt = pool.tile([Q, O], MM_DT, tag=f"g_{name}", name=f"g_{name}")
                nc.vector.tensor_copy(gt[:], ps_g[:] if MM_DT == F16 else ps_g[:].bitcast(F32R))
                g_sb.append(gt)

            # ---- P tiles (per-output-row broadcast, [o, i]) ----
            p_sb = []
            for name, psel in (("pa", pselA), ("pb", pselB)):
                ps_p = ppaux.tile([O, Q], F32, tag="cm", name=f"psp_{name}")
                for k, s in enumerate(srcs):
                    nc.tensor.matmul(ps_p[:], psel[:, k * O:(k + 1) * O], s,
                                     start=(k == 0), stop=(k == 7))
                pt = pool.tile([O, Q], F32, tag=f"p_{name}", name=f"p_{name}")
                nc.vector.tensor_copy(pt[:], ps_p[:])
                p_sb.append(pt)

            # ---- main loop: 2 quads x 32 groups of 4 rows (N=512) ----
            w0 = w0m[:]
            quads = [(rtA, rtB, g_sb[0], p_sb[0], outA, "A"),
                     (rtB, rtA, g_sb[1], p_sb[1], outB, "B")]
            for rt_self, rt_other, gt, pt, out_t, qn in quads:
                v_self = rt_self[:].rearrange("c (j i) -> c i j", j=Q, i=Q)
                v_other = rt_other[:].rearrange("c (j i) -> c i j", j=Q, i=Q)
                for grp in range(8):  # 4 groups of 4 rows -> [O, 2048] stage
                    stage = stpool.tile([O, 8 * 256], F32, tag="stage",
                                        name=f"st{qn}{grp}")
                    for sub in range(4):
                        t4 = grp * 4 + sub
                        ps = ppmain.tile([O, 512], F32, tag="main",
                                         name=f"m{qn}{grp}_{sub}")
                        nc.tensor.matmul(ps[:], w0,
                                         rt_self[:, t4 * 512:(t4 + 1) * 512],
                                         start=True, stop=False)
                        nc.tensor.matmul(ps[:], w1a,
                                         v_self[:, 4 * t4:4 * t4 + 4, :],
                                         start=False, stop=False)
                        nc.tensor.matmul(ps[:], w1b,
                                         v_other[:, 4 * t4:4 * t4 + 4, :],
                                         start=False, stop=False)
                        nc.tensor.matmul(ps[:], gt[:], i2t[:],
                                         start=False, stop=True)
                        for r in range(4):
                            off = sub * 512 + r * Q
                            i_loc = 4 * t4 + r
                            if r < 2:
                                nc.vector.tensor_scalar(
                                    stage[:, off:off + Q], ps[:, r * Q:(r + 1) * Q],
                                    pt[:, i_loc:i_loc + 1], None, op0=ADD)
                            else:
                                nc.scalar.activation(
                                    stage[:, off:off + Q], ps[:, r * Q:(r + 1) * Q],
                                    mybir.ActivationFunctionType.Identity,
                                    bias=pt[:, i_loc:i_loc + 1], scale=1.0)
                    nc.sync.dma_start(out_t[:, grp * 2048:(grp + 1) * 2048],
                                      stage[:])
    return nc


def _get_nc(use_collective=True):
    key = use_collective
    if key not in _NC_CACHE:
        nc = build(use_collective)
        nc.finalize()
        _NC_CACHE[key] = nc
    return _NC_CACHE[key]


def _host_prep(E, W, bias):
    """Build per-core in_maps from full inputs."""
    Wt = np.ascontiguousarray(W.transpose(1, 0, 2))  # [c, k, o]
    Wt = Wt.copy()
    Wt[:, 6, :] *= 0.5  # W6 consumed against 2*sa
    wsb = Wt.reshape(C, 7 * O)
    eye = np.eye(Q, dtype=np.float32)
    i2 = np.concatenate([eye, eye, eye, eye], axis=1).astype(E_NP)
    biasr = bias.reshape(1, O).astype(np.float32)
    ones = np.ones((1, Q), dtype=np.float32)
    W1, W2, W3, W4, W5 = W[1], W[2], W[3], W[4], W[5]
    Z = np.zeros_like(W1)

    in_maps = []
    for core in range(N_CORES):
        b, h = core // 2, core % 2
        # quads: qA = (0, h), qB = (1, 1-h)
        qA = (0, h)
        qB = (1, 1 - h)

        def quad(pq):
            p, q = pq
            blk = E[b, p * Q:(p + 1) * Q, q * Q:(q + 1) * Q, :]
            return np.ascontiguousarray(blk.transpose(2, 0, 1)).reshape(C, QF).astype(E_NP)

        # transpose-term selectors: quad (p,q) needs E(q,p); for h=0 that is
        # the same tile ("self"), for h=1 the sibling tile ("other")
        w1a_ = _sel(W1, h == 0).astype(E_NP)
        w1b_ = _sel(W1, h == 1).astype(E_NP)

        # source slots: (rank r, X in {A,B}, kind in {pr, pcm})
        # quad of (r, A) = (0, r); quad of (r, B) = (1, 1-r)
        def gsel_for(colset):
            parts = []
            for r in range(2):
                for X in range(2):  # 0=A, 1=B
                    rows = 0 if X == 0 else 1
                    qcol = r if X == 0 else 1 - r
                    parts.append(_sel(W4, rows == colset))   # pr slot
                    parts.append(_sel(W3, qcol == colset))   # pcm slot
            return np.concatenate(parts, axis=1).astype(np.float32)

        def psel_for(rowset):
            parts = []
            for r in range(2):
                for X in range(2):
                    rows = 0 if X == 0 else 1
                    qcol = r if X == 0 else 1 - r
                    parts.append(_sel(W2, rows == rowset))   # pr slot
                    parts.append(_sel(W5, qcol == rowset))   # pcm slot
            return np.concatenate(parts, axis=1).astype(np.float32)

        # wait: gsel slices are RHS [c, o] per source in order
        # (r0A_pr, r0A_pcm, r0B_pr, r0B_pcm, r1A_pr, r1A_pcm, r1B_pr, r1B_pcm)
        gselA_ = gsel_for(h)        # G for qA's colset (= h)
        gselB_ = gsel_for(1 - h)    # G for qB's colset
        pselA_ = psel_for(0)        # P for qA's rowset (= 0)
        pselB_ = psel_for(1)        # P for qB's rowset (= 1)

        in_maps.append({
            "eqA": quad(qA), "eqB": quad(qB), "wsb": wsb, "i2": i2,
            "w0m": W[0].astype(E_NP),
            "w1a": w1a_, "w1b": w1b_, "gselA": gselA_, "gselB": gselB_,
            "pselA": pselA_, "pselB": pselB_, "biasr": biasr, "ones": ones,
        })
    return in_maps


def _unshard(results, dtype):
    out = np.empty((B, N, N, O), dtype=dtype)
    for core in range(N_CORES):
        b, h = core // 2, core % 2
        for name, (p, q) in (("outA", (0, h)), ("outB", (1, 1 - h))):
            arr = results[core][name].reshape(O, Q, Q)  # [o, i, j]
            out[b, p * Q:(p + 1) * Q, q * Q:(q + 1) * Q, :] = \
                arr.transpose(1, 2, 0)
    return out


def kernel(x=None, adj=None, edge_attrs=None, W=None, bias=None, **_):
    E = np.asarray(edge_attrs, dtype=np.float32)
    Wf = np.asarray(W, dtype=np.float32)
    bf = np.asarray(bias, dtype=np.float32)
    in_maps = _host_prep(E, Wf, bf)
    nc = _get_nc(use_collective=True)
    res = run_bass_kernel_spmd(nc, in_maps, core_ids=list(range(N_CORES)))
    return _unshard(res.results, np.float32)


# revision 6
# speedup vs baseline: 1.5305x; 1.5305x over previous
"""Trainium2 Bass kernel for nn_DenseGraphConvEdgeToEdge (B=4, N=256, C=O=128).

out[b,i,j,:] = E[b,i,j]@W0 + E[b,j,i]@W1 + R[b,i]@W2 + Cm[b,j]@W3
             + R[b,j]@W4 + Cm[b,i]@W5 + sa[b]@W6 + bias
where R = E.sum(axis=2) (row sums), Cm = E.sum(axis=1) (col sums),
sa = E.sum(axis=(1,2)).

Sharding: 8 cores = 4 batches x 2 halves. Core (b, h) owns output quadrants
qA=(0,h), qB=(1,1-h) (quadrant (p,q) = rows p*128:(p+1)*128 x cols
q*128:(q+1)*128). For each output quadrant the host ships the E-quadrant it
needs twice, in fp16: once i-major ([c, i*128+j], feeding the E@W0 term) and
once j-major (the transpose-partner quadrant pre-transposed, feeding the
E^T@W1 term) -- so every tensor-engine stream is contiguous and the program
is SPMD-uniform with all per-core routing decided by host data placement.

Marginals: per-tile column sums via identity-matmul PSUM accumulation (the
j-major tiles' column sums are the row sums R). Each core exchanges its
4 partial-marginal vectors with its batch sibling via a pairwise AllGather
(the only collective), then forms the broadcast terms G (per output column,
includes sa@W6 + bias) and P (per output row) with matmuls against
host-built per-core selector weight tables. G is folded into the PSUM
accumulation via an [I|I|I|I] identity matmul; P is added during the
PSUM->SBUF drain (per-partition scalar add on DVE/ACT).

Main matmuls are fp16 (input quantization ~5e-4 relative); the small
marginal/broadcast matmuls run as float32r. End-to-end rel err ~3e-4.
"""
import numpy as np

import concourse.mybir as mybir
import concourse.tile as tile
from concourse import bacc
from concourse.bass_utils import run_bass_kernel_spmd

F32 = mybir.dt.float32
F32R = mybir.dt.float32r
F16 = mybir.dt.float16
ADD = mybir.AluOpType.add
E_NP = np.float16

B, N, C, O = 4, 256, 128, 128
Q = 128          # quadrant side
QF = Q * Q       # quadrant flat free size
N_CORES = 8

_NC_CACHE = {}


def _sel(w, cond):
    return w if cond else np.zeros_like(w)


def build(use_collective=True):
    nc = bacc.Bacc(trn_type="TRN2")

    # per-core inputs (all fp16 E data; f32 weights/selectors)
    eqA = nc.dram_tensor("eqA", [C, QF], F16, kind="ExternalInput")
    eqB = nc.dram_tensor("eqB", [C, QF], F16, kind="ExternalInput")
    tqA = nc.dram_tensor("tqA", [C, QF], F16, kind="ExternalInput")
    tqB = nc.dram_tensor("tqB", [C, QF], F16, kind="ExternalInput")
    w0_d = nc.dram_tensor("w0m", [C, O], F16, kind="ExternalInput")
    w1_d = nc.dram_tensor("w1m", [C, O], F16, kind="ExternalInput")
    i2_d = nc.dram_tensor("i2", [C, 4 * Q], F16, kind="ExternalInput")
    wsb_d = nc.dram_tensor("wsb", [C, 7 * O], F32, kind="ExternalInput")
    gselA_d = nc.dram_tensor("gselA", [C, 8 * O], F32, kind="ExternalInput")
    gselB_d = nc.dram_tensor("gselB", [C, 8 * O], F32, kind="ExternalInput")
    pselA_d = nc.dram_tensor("pselA", [C, 8 * O], F32, kind="ExternalInput")
    pselB_d = nc.dram_tensor("pselB", [C, 8 * O], F32, kind="ExternalInput")
    biasr_d = nc.dram_tensor("biasr", [1, O], F32, kind="ExternalInput")
    ones_d = nc.dram_tensor("ones", [1, Q], F32, kind="ExternalInput")
    outA = nc.dram_tensor("outA", [O, QF], F32, kind="ExternalOutput")
    outB = nc.dram_tensor("outB", [O, QF], F32, kind="ExternalOutput")

    with tile.TileContext(nc) as tc:
        with (
            tc.tile_pool(name="pool", bufs=1) as pool,
            tc.tile_pool(name="stpool", bufs=3) as stpool,
            tc.tile_pool(name="ppmain", bufs=5, space="PSUM") as ppmain,
            tc.tile_pool(name="ppaux", bufs=3, space="PSUM") as ppaux,
            tc.tile_pool(name="dram", bufs=1, space="DRAM") as dram,
        ):
            # ---- small constant loads ----
            wsb = pool.tile([C, 7 * O], F32R, tag="wsb")
            nc.sync.dma_start(wsb[:], wsb_d[:].bitcast(F32R))
            i2t = pool.tile([C, 4 * Q], F16, tag="i2t")
            nc.sync.dma_start(i2t[:], i2_d[:])
            w0m = pool.tile([C, O], F16, tag="w0m")
            nc.sync.dma_start(w0m[:], w0_d[:])
            w1m = pool.tile([C, O], F16, tag="w1m")
            nc.sync.dma_start(w1m[:], w1_d[:])
            gselA = pool.tile([C, 8 * O], F32R, tag="gselA")
            nc.sync.dma_start(gselA[:], gselA_d[:].bitcast(F32R))
            gselB = pool.tile([C, 8 * O], F32R, tag="gselB")
            nc.sync.dma_start(gselB[:], gselB_d[:].bitcast(F32R))
            pselA = pool.tile([C, 8 * O], F32R, tag="pselA")
            nc.sync.dma_start(pselA[:], pselA_d[:].bitcast(F32R))
            pselB = pool.tile([C, 8 * O], F32R, tag="pselB")
            nc.sync.dma_start(pselB[:], pselB_d[:].bitcast(F32R))
            biasr = pool.tile([1, O], F32, tag="biasr")
            nc.sync.dma_start(biasr[:], biasr_d[:])
            onesr = pool.tile([1, Q], F32R, tag="onesr")
            nc.sync.dma_start(onesr[:], ones_d[:].bitcast(F32R))

            # ---- resident E tiles (2 quads x 2 layouts), chunked loads ----
            rtA = pool.tile([C, QF], F16, tag="rtA")
            rtB = pool.tile([C, QF], F16, tag="rtB")
            vtA = pool.tile([C, QF], F16, tag="vtA")
            vtB = pool.tile([C, QF], F16, tag="vtB")
            NCHUNK = 4
            CH = QF // NCHUNK  # 4096 (1 MiB fp16 per chunk DMA)
            tiles_srcs = ((rtA, eqA), (vtA, tqA), (rtB, eqB), (vtB, tqB))
            for rt, src in tiles_srcs:
                for k in range(NCHUNK):
                    sl = slice(k * CH, (k + 1) * CH)
                    nc.sync.dma_start(rt[:, sl], src[:, sl])

            # ---- per-tile column-sum partials (identity-matmul accum) ----
            # pack slots: [cs(vtA) | cs(rtA) | cs(vtB) | cs(rtB)]
            # (cs(vtX) are row sums of the W1-source quadrant)
            own_pack = pool.tile([C, 512], F32, tag="own_pack")
            ident = i2t[:, 0:Q]
            for si, rt in enumerate((vtA, rtA, vtB, rtB)):
                ps_cm = ppaux.tile([C, 2 * Q], F32, tag="cm", name=f"pscm{si}")
                for t in range(Q // 2):
                    nc.tensor.matmul(ps_cm[:], ident, rt[:, t * 256:(t + 1) * 256],
                                     start=(t == 0), stop=(t == Q // 2 - 1))
                tmp = pool.tile([C, Q], F32, tag="cmtmp", name=f"cmtmp{si}")
                nc.vector.tensor_copy(tmp[:], ps_cm[:, 0:Q])
                nc.vector.tensor_tensor(own_pack[:, si * Q:(si + 1) * Q],
                                        tmp[:], ps_cm[:, Q:2 * Q], op=ADD)

            # ---- exchange partials with batch sibling ----
            cc_in = dram.tile([C, 512], F32, tag="cc_in")
            cc_out = dram.tile([2 * C, 512], F32, tag="cc_out")
            nc.gpsimd.dma_start(cc_in[:], own_pack[:])
            if use_collective:
                nc.gpsimd.collective_compute(
                    "AllGather", mybir.AluOpType.bypass,
                    replica_groups=[[0, 1], [2, 3], [4, 5], [6, 7]],
                    ins=[cc_in[:].opt()], outs=[cc_out[:].opt()])
            else:
                nc.gpsimd.dma_start(cc_out[0:C, :], cc_in[:])
                nc.gpsimd.dma_start(cc_out[C:2 * C, :], cc_in[:])
            pk0 = pool.tile([C, 512], F32R, tag="pk0")
            nc.sync.dma_start(pk0[:], cc_out[0:C, :].bitcast(F32R))
            pk1 = pool.tile([C, 512], F32R, tag="pk1")
            nc.sync.dma_start(pk1[:], cc_out[C:2 * C, :].bitcast(F32R))

            # ---- sa (sum over everything); pack totals = 2*sa ----
            sa0 = pool.tile([C, 1], F32, tag="sa0")
            nc.vector.tensor_reduce(sa0[:], pk0[:].bitcast(F32),
                                    axis=mybir.AxisListType.X, op=ADD)
            sa1 = pool.tile([C, 1], F32, tag="sa1")
            nc.vector.tensor_reduce(sa1[:], pk1[:].bitcast(F32),
                                    axis=mybir.AxisListType.X, op=ADD)
            sa2 = pool.tile([C, 1], F32, tag="sa2")
            nc.vector.tensor_tensor(sa2[:], sa0[:], sa1[:], op=ADD)
            saT = pool.tile([C, 1], F32R, tag="saT")
            nc.vector.tensor_copy(saT[:], sa2[:].bitcast(F32R))

            # ---- s = (2*sa) @ (W6/2) + bias, as a [1, O] row ----
            ps_s = ppaux.tile([1, O], F32, tag="cm", name="ps_s")
            nc.tensor.matmul(ps_s[:], saT[:], wsb[:, 6 * O:7 * O],
                             start=True, stop=True)
            sbrow = pool.tile([1, O], F32, tag="sbrow")
            nc.vector.tensor_tensor(sbrow[:], biasr[:], ps_s[:], op=ADD)
            sbrow_r = pool.tile([1, O], F32R, tag="sbrow_r")
            nc.vector.tensor_copy(sbrow_r[:], sbrow[:].bitcast(F32R))

            # ---- G tiles (per-output-column broadcast, [j, o]) ----
            srcs = [pk0[:, k * Q:(k + 1) * Q] for k in range(4)] + \
                   [pk1[:, k * Q:(k + 1) * Q] for k in range(4)]
            g_sb = []
            for name, gsel in (("ga", gselA), ("gb", gselB)):
                ps_g = ppaux.tile([Q, O], F32, tag="cm", name=f"psg_{name}")
                for k, s in enumerate(srcs):
                    nc.tensor.matmul(ps_g[:], s, gsel[:, k * O:(k + 1) * O],
                                     start=(k == 0), stop=False)
                nc.tensor.matmul(ps_g[:], onesr[:], sbrow_r[:],
                                 start=False, stop=True)
                gt = pool.tile([Q, O], F16, tag=f"g_{name}", name=f"g_{name}")
                nc.vector.tensor_copy(gt[:], ps_g[:])
                g_sb.append(gt)

            # ---- P tiles (per-output-row broadcast, [o, i]) ----
            p_sb = []
            for name, psel in (("pa", pselA), ("pb", pselB)):
                ps_p = ppaux.tile([O, Q], F32, tag="cm", name=f"psp_{name}")
                for k, s in enumerate(srcs):
                    nc.tensor.matmul(ps_p[:], psel[:, k * O:(k + 1) * O], s,
                                     start=(k == 0), stop=(k == 7))
                pt = pool.tile([O, Q], F32, tag=f"p_{name}", name=f"p_{name}")
                nc.vector.tensor_copy(pt[:], ps_p[:])
                p_sb.append(pt)

            # ---- main loop: 2 quads x 32 groups of 4 rows (N=512) ----
            quads = [(rtA, vtA, g_sb[0], p_sb[0], outA, "A"),
                     (rtB, vtB, g_sb[1], p_sb[1], outB, "B")]
            for rt_self, vt_self, gt, pt, out_t, qn in quads:
                for grp in range(8):  # 4 groups of 4 rows -> [O, 2048] stage
                    stage = stpool.tile([O, 8 * 256], F32, tag="stage",
                                        name=f"st{qn}{grp}")
                    for sub in range(4):
                        t4 = grp * 4 + sub
                        sl = slice(t4 * 512, (t4 + 1) * 512)
                        ps = ppmain.tile([O, 512], F32, tag="main",
                                         name=f"m{qn}{grp}_{sub}")
                        nc.tensor.matmul(ps[:], w0m[:], rt_self[:, sl],
                                         start=True, stop=False)
                        nc.tensor.matmul(ps[:], w1m[:], vt_self[:, sl],
                                         start=False, stop=False)
                        nc.tensor.matmul(ps[:], gt[:], i2t[:],
                                         start=False, stop=True)
                        for r in range(4):
                            off = sub * 512 + r * Q
                            i_loc = 4 * t4 + r
                            if r < 2:
                                nc.vector.tensor_scalar(
                                    stage[:, off:off + Q], ps[:, r * Q:(r + 1) * Q],
                                    pt[:, i_loc:i_loc + 1], None, op0=ADD)
                            else:
                                nc.scalar.activation(
                                    stage[:, off:off + Q], ps[:, r * Q:(r + 1) * Q],
                                    mybir.ActivationFunctionType.Identity,
                                    bias=pt[:, i_loc:i_loc + 1], scale=1.0)
                    nc.sync.dma_start(out_t[:, grp * 2048:(grp + 1) * 2048],
                                      stage[:])
    return nc


def _get_nc(use_collective=True):
    key = use_collective
    if key not in _NC_CACHE:
        nc = build(use_collective)
        nc.finalize()
        _NC_CACHE[key] = nc
    return _NC_CACHE[key]


def _host_prep(E, W, bias):
    """Build per-core in_maps from full inputs."""
    Wt = np.ascontiguousarray(W.transpose(1, 0, 2))  # [c, k, o]
    Wt = Wt.copy()
    Wt[:, 6, :] *= 0.5  # W6 consumed against 2*sa
    wsb = Wt.reshape(C, 7 * O)
    eye = np.eye(Q, dtype=np.float32)
    i2 = np.concatenate([eye, eye, eye, eye], axis=1).astype(E_NP)
    biasr = bias.reshape(1, O).astype(np.float32)
    ones = np.ones((1, Q), dtype=np.float32)
    W2, W3, W4, W5 = W[2], W[3], W[4], W[5]

    in_maps = []
    for core in range(N_CORES):
        b, h = core // 2, core % 2

        def quad_i(p, q):
            # i-major: [c, i*128+j] of quadrant (p, q)
            blk = E[b, p * Q:(p + 1) * Q, q * Q:(q + 1) * Q, :]
            return np.ascontiguousarray(
                blk.transpose(2, 0, 1)).reshape(C, QF).astype(E_NP)

        def quad_j(p, q):
            # j-major transpose source: [c, i*128+j] = E-quad(p,q)[j, i]
            blk = E[b, p * Q:(p + 1) * Q, q * Q:(q + 1) * Q, :]
            return np.ascontiguousarray(
                blk.transpose(2, 1, 0)).reshape(C, QF).astype(E_NP)

        # out-quad qA = (0, h): W0 source = quad (0, h); W1 source =
        # quad (h, 0) transposed. out-quad qB = (1, 1-h): W0 = (1, 1-h);
        # W1 = (1-h, 1) transposed.
        eqA_ = quad_i(0, h)
        eqB_ = quad_i(1, 1 - h)
        tqA_ = quad_j(h, 0)
        tqB_ = quad_j(1 - h, 1)

        # source slots after the pairwise AllGather, per rank r:
        #   slot0 = cs(vtA of rank r) = row sums of quad (r, 0)
        #   slot1 = cs(rtA)           = col sums of quad (0, r)
        #   slot2 = cs(vtB)           = row sums of quad (1-r, 1)
        #   slot3 = cs(rtB)           = col sums of quad (1, 1-r)
        # pr-type slot with quad (p,q) covers R-block p (partial over cols q)
        # pcm-type slot with quad (p,q) covers Cm-block q (partial over rows p)
        slot_quads = []
        for r in range(2):
            slot_quads += [((r, 0), "pr"), ((0, r), "pcm"),
                           ((1 - r, 1), "pr"), ((1, 1 - r), "pcm")]

        def gsel_for(colset):
            parts = []
            for (p, q), kind in slot_quads:
                if kind == "pr":
                    parts.append(_sel(W4, p == colset))
                else:
                    parts.append(_sel(W3, q == colset))
            return np.concatenate(parts, axis=1).astype(np.float32)

        def psel_for(rowset):
            parts = []
            for (p, q), kind in slot_quads:
                if kind == "pr":
                    parts.append(_sel(W2, p == rowset))
                else:
                    parts.append(_sel(W5, q == rowset))
            return np.concatenate(parts, axis=1).astype(np.float32)

        in_maps.append({
            "eqA": eqA_, "eqB": eqB_, "tqA": tqA_, "tqB": tqB_,
            "w0m": W[0].astype(E_NP), "w1m": W[1].astype(E_NP),
            "wsb": wsb, "i2": i2,
            "gselA": gsel_for(h), "gselB": gsel_for(1 - h),
            "pselA": psel_for(0), "pselB": psel_for(1),
            "biasr": biasr, "ones": ones,
        })
    return in_maps


def _unshard(results, dtype):
    out = np.empty((B, N, N, O), dtype=dtype)
    for core in range(N_CORES):
        b, h = core // 2, core % 2
        for name, (p, q) in (("outA", (0, h)), ("outB", (1, 1 - h))):
            arr = results[core][name].reshape(O, Q, Q)  # [o, i, j]
            out[b, p * Q:(p + 1) * Q, q * Q:(q + 1) * Q, :] = \
                arr.transpose(1, 2, 0)
    return out


def kernel(x=None, adj=None, edge_attrs=None, W=None, bias=None, **_):
    E = np.asarray(edge_attrs, dtype=np.float32)
    Wf = np.asarray(W, dtype=np.float32)
    bf = np.asarray(bias, dtype=np.float32)
    in_maps = _host_prep(E, Wf, bf)
    nc = _get_nc(use_collective=True)
    res = run_bass_kernel_spmd(nc, in_maps, core_ids=list(range(N_CORES)))
    return _unshard(res.results, np.float32)
